# revision 12
# baseline (speedup 1.0000x reference)
"""Trainium2 Bass kernel for nn_MeshNodeBlock (GNN message passing node block).

reference:
    agg = segment_sum(efeat, dst_idx, N)           # [N, 512]
    cat = concat([agg, nfeat], -1)                 # [N, 1024]
    h   = silu(cat @ w1 + b1)                      # [N, 512]
    y   = layernorm(h @ w2 + b2) * gamma + beta    # [N, 512]
    out = (efeat, y + nfeat)

Strategy (no collectives needed):
  * Host packs the 25000 nodes into 200 groups of <=128 nodes with near-equal
    edge counts (greedy bin packing), permutes nfeat accordingly and lays the
    edges out per-group, padded to a uniform CPG chunks of 128 edges.
  * Each of the 8 cores handles 25 groups.  Per 128-edge chunk the kernel
    builds a one-hot [128e x 128n] matrix on DVE (iota == dst_local); the PE
    computes agg[n, f] += onehot[e, n]^T efeat_chunk[e, f] with the one-hot
    stationary (one N=512 matmul per chunk), accumulating in PSUM.
  * agg is transposed via TensorE (4x 128x128) into catT[feat, node]; nfeatT
    arrives host-pre-transposed.  MLP: W1 tiles stationary over catT -> PSUM
    H^T; sigmoid(x+b1)*(x+b1) for SiLU; second matmul with H^T stationary
    over W2 produces y in natural [node, feat] layout; LayerNorm along the
    free dim; residual add; store.
  * Matmul operands are bf16 (4x PE rate vs fp32 + half DMA); all
    accumulation, LayerNorm and the residual stay fp32.  Set
    PRECISION="fp32" for a full-precision (4x slower) variant.
"""

import heapq
import time
from contextlib import ExitStack

import numpy as np
import ml_dtypes

N_NODES = 25000
N_EDGES = 400000
D = 512
P = 128
N_CORES = 8
N_GROUPS = 200  # 25 groups per core
LN_EPS = 1e-5
DMA_CHUNK_BATCH = 16
PRECISION = "bf16"  # "bf16" | "fp32"

_np_bf16 = ml_dtypes.bfloat16


# ---------------------------------------------------------------- host packing
def pack_inputs(efeat, nfeat, dst_idx, w1, b1, w2, b2, gamma, beta,
                n_groups=N_GROUPS, min_cpg=1, precision=None):
    """Bin-pack nodes into groups, permute/pad edges + node data.

    Returns (per_core_inputs: list[dict], perm, cpg, gpc).
    """
    precision = precision or PRECISION
    np_mm = _np_bf16 if precision == "bf16" else np.float32
    n_nodes = nfeat.shape[0]
    n_edges = efeat.shape[0]
    d = efeat.shape[1]
    assert n_groups % N_CORES == 0
    gpc = n_groups // N_CORES
    slots = n_groups * P
    assert slots >= n_nodes

    deg = np.bincount(dst_idx, minlength=n_nodes).astype(np.int64)
    order = np.argsort(-deg, kind="stable")

    node_group = np.empty(n_nodes, np.int32)
    node_local = np.empty(n_nodes, np.int32)
    group_count = np.zeros(n_groups, np.int32)
    group_load = np.zeros(n_groups, np.int64)
    heap = [(0, g) for g in range(n_groups)]
    heapq.heapify(heap)
    for nid in order:
        while True:
            load, g = heapq.heappop(heap)
            if group_count[g] < P:
                break
        node_group[nid] = g
        node_local[nid] = group_count[g]
        group_count[g] += 1
        group_load[g] += deg[nid]
        heapq.heappush(heap, (int(group_load[g]), g))

    cpg = max(min_cpg, int(-(-int(group_load.max()) // P)))  # ceil
    gsz = cpg * P  # edge slots per group

    # slot -> node id permutation
    perm = np.full(slots, -1, np.int64)
    perm[node_group.astype(np.int64) * P + node_local] = np.arange(n_nodes)

    # edges: stable-sort by group, place into padded per-group ranges
    eg = node_group[dst_idx]
    el = node_local[dst_idx].astype(np.float32)
    eorder = np.argsort(eg, kind="stable")
    counts = np.bincount(eg, minlength=n_groups)
    starts = np.concatenate([[0], np.cumsum(counts)[:-1]])
    eg_sorted = eg[eorder]
    pos = np.arange(n_edges) - starts[eg_sorted]
    slot = eg_sorted.astype(np.int64) * gsz + pos

    EF = np.zeros((n_groups * gsz, d), np_mm)
    EF[slot] = efeat[eorder].astype(np_mm)
    DL = np.full(n_groups * gsz, -1.0, np.float32)
    DL[slot] = el[eorder]

    # permuted node features
    nfeat_p = np.zeros((slots, d), np.float32)
    valid = perm >= 0
    nfeat_p[valid] = nfeat[perm[valid]]
    # nfeatT tiles: [n_groups, 4, 128, 128]  tile[g,kt,p,j] = nfeat_p[g*128+j, kt*128+p]
    NFT = np.ascontiguousarray(
        nfeat_p.reshape(n_groups, P, d // P, P).transpose(0, 2, 3, 1)).astype(np_mm)
    RES = nfeat_p + beta[None, :].astype(np.float32)

    W1S = np.ascontiguousarray(
        w1.reshape(w1.shape[0] // P, P, w1.shape[1])).astype(np_mm)
    W2S = np.ascontiguousarray(
        w2.reshape(w2.shape[0] // P, P, w2.shape[1])).astype(np_mm)
    B1 = np.ascontiguousarray(b1.reshape(d // P, P).T).astype(np.float32)
    B2 = np.ascontiguousarray(b2.reshape(1, d)).astype(np_mm)
    GAM = np.ascontiguousarray(np.broadcast_to(gamma, (P, d))).astype(np.float32)
    IOTA = np.ascontiguousarray(
        np.broadcast_to(np.arange(P, dtype=np.float32), (P, P)))
    IDENT = np.eye(P, dtype=np_mm)

    per_core = []
    for c in range(N_CORES):
        g0 = c * gpc
        dl_c = DL[g0 * gsz:(g0 + gpc) * gsz]
        per_core.append({
            "EF": EF[g0 * gsz:(g0 + gpc) * gsz],
            "DSTL": np.ascontiguousarray(dl_c.reshape(gpc * cpg, P).T),
            "NFT": NFT[g0:g0 + gpc],
            "RES": RES[g0 * P:(g0 + gpc) * P],
            "W1S": W1S, "W2S": W2S, "B1": B1, "B2": B2, "GAM": GAM,
            "IOTA": IOTA, "IDENT": IDENT,
        })
    return per_core, perm, cpg, gpc


# ---------------------------------------------------------------- bass program
def build_nc(gpc, cpg, d=D, reps=1, mode="full", precision=None):
    import concourse.bass as bass
    import concourse.tile as tile
    from concourse import bacc, mybir

    precision = precision or PRECISION
    f32 = mybir.dt.float32
    fmm = mybir.dt.bfloat16 if precision == "bf16" else f32
    FT = d // P  # feature tiles (4)
    KT = 2 * FT  # cat k-tiles (8)
    gsz = cpg * P

    nc = bacc.Bacc("TRN2", target_bir_lowering=False, debug=False,
                   num_devices=N_CORES)

    EF = nc.dram_tensor("EF", [gpc * gsz, d], fmm, kind="ExternalInput").ap()
    DSTL = nc.dram_tensor("DSTL", [P, gpc * cpg], f32, kind="ExternalInput").ap()
    NFT = nc.dram_tensor("NFT", [gpc, FT, P, P], fmm, kind="ExternalInput").ap()
    RES = nc.dram_tensor("RES", [gpc * P, d], f32, kind="ExternalInput").ap()
    W1S = nc.dram_tensor("W1S", [KT, P, d], fmm, kind="ExternalInput").ap()
    W2S = nc.dram_tensor("W2S", [FT, P, d], fmm, kind="ExternalInput").ap()
    B1 = nc.dram_tensor("B1", [P, FT], f32, kind="ExternalInput").ap()
    B2 = nc.dram_tensor("B2", [1, d], fmm, kind="ExternalInput").ap()
    GAM = nc.dram_tensor("GAM", [P, d], f32, kind="ExternalInput").ap()
    IOTA = nc.dram_tensor("IOTA", [P, P], f32, kind="ExternalInput").ap()
    IDENT = nc.dram_tensor("IDENT", [P, P], fmm, kind="ExternalInput").ap()
    OUT = nc.dram_tensor("OUT", [gpc * P, d], f32, kind="ExternalOutput").ap()

    eq = mybir.AluOpType.is_equal
    mult = mybir.AluOpType.mult
    add = mybir.AluOpType.add
    AF = mybir.ActivationFunctionType

    with tile.TileContext(nc) as tc, ExitStack() as ctx:
        const_pool = ctx.enter_context(tc.tile_pool(name="const", bufs=1))
        ef_pool = ctx.enter_context(tc.tile_pool(name="ef", bufs=4))
        oh_pool = ctx.enter_context(tc.tile_pool(name="oh", bufs=16))
        agg_sb_pool = ctx.enter_context(tc.tile_pool(name="aggsb", bufs=2))
        cat_pool = ctx.enter_context(tc.tile_pool(name="cat", bufs=2))
        h_pool = ctx.enter_context(tc.tile_pool(name="h", bufs=2))
        big_pool = ctx.enter_context(tc.tile_pool(name="big", bufs=2))
        stat_pool = ctx.enter_context(tc.tile_pool(name="stat", bufs=3))
        agg_ps = ctx.enter_context(
            tc.tile_pool(name="aggps", bufs=2, space="PSUM"))
        trp_ps = ctx.enter_context(
            tc.tile_pool(name="trpps", bufs=2, space="PSUM"))
        h_ps = ctx.enter_context(tc.tile_pool(name="hps", bufs=2, space="PSUM"))
        y_ps = ctx.enter_context(tc.tile_pool(name="yps", bufs=2, space="PSUM"))

        # persistent constants
        w1_sb = const_pool.tile([P, KT, d], fmm)
        for k in range(KT):
            nc.sync.dma_start(w1_sb[:, k, :], W1S[k])
        w2_sb = const_pool.tile([P, FT, d], fmm)
        for k in range(FT):
            nc.sync.dma_start(w2_sb[:, k, :], W2S[k])
        b1_sb = const_pool.tile([P, FT], f32)
        nc.sync.dma_start(b1_sb[:], B1[:])
        b2_sb = const_pool.tile([1, d], fmm)
        nc.sync.dma_start(b2_sb[:], B2[:])
        gam_sb = const_pool.tile([P, d], f32)
        nc.sync.dma_start(gam_sb[:], GAM[:])
        iota_sb = const_pool.tile([P, P], f32)
        nc.sync.dma_start(iota_sb[:], IOTA[:])
        ident_sb = const_pool.tile([P, P], fmm)
        nc.sync.dma_start(ident_sb[:], IDENT[:])
        dstl_sb = const_pool.tile([P, gpc * cpg], f32)
        nc.sync.dma_start(dstl_sb[:], DSTL[:])
        ones_sb = const_pool.tile([1, P], fmm)
        nc.vector.memset(ones_sb[:], 1.0)
        eps_sb = const_pool.tile([P, 1], f32)
        nc.vector.memset(eps_sb[:], LN_EPS)

        def group_body(g):
            eg_base = g * gsz
            # ---- load efeat chunks (batched DMAs)
            ef_tiles = []
            for b0 in range(0, cpg, DMA_CHUNK_BATCH):
                nb = min(DMA_CHUNK_BATCH, cpg - b0)
                t = ef_pool.tile([P, nb, d], fmm, tag="ef")
                src = EF[eg_base + b0 * P: eg_base + (b0 + nb) * P, :]
                nc.sync.dma_start(t[:], src.rearrange("(c p) f -> p c f", p=P))
                ef_tiles.append((t, nb))

            catT = cat_pool.tile([P, KT, P], fmm)
            nc.sync.dma_start(catT[:, FT:KT, :],
                              NFT[g].rearrange("k p j -> p k j"))

            if mode in ("full", "segsum"):
                # ---- segment-sum: agg[n, f] += oh[e, n]^T @ ef[e, f]
                agg = agg_ps.tile([P, d], f32)
                ci = 0
                for t, nb in ef_tiles:
                    for c in range(nb):
                        oh = oh_pool.tile([P, P], fmm)
                        col = g * cpg + ci
                        nc.vector.tensor_scalar(
                            oh[:], iota_sb[:], dstl_sb[:, col:col + 1], None,
                            eq)
                        nc.tensor.matmul(agg[:], oh[:], t[:, c, :],
                                         start=(ci == 0),
                                         stop=(ci == cpg - 1))
                        ci += 1
                # ---- transpose agg into catT[0:FT]
                agg_sb = agg_sb_pool.tile([P, d], fmm)
                nc.scalar.copy(agg_sb[:], agg[:])
                for ft in range(FT):
                    trp = trp_ps.tile([P, P], fmm)
                    nc.tensor.transpose(trp[:], agg_sb[:, ft * P:(ft + 1) * P],
                                        ident_sb[:])
                    nc.scalar.copy(catT[:, ft, :], trp[:])
            elif mode == "noseg":
                for ft in range(FT):
                    nc.scalar.copy(catT[:, ft, :], catT[:, FT + ft, :])
                for t, nb in ef_tiles:
                    nc.scalar.copy(catT[:, 0:1, 0:1], t[:, 0:1, 0:1])
            elif mode == "dma":
                for t, nb in ef_tiles:
                    nc.scalar.copy(catT[:, 0:1, 0:1], t[:, 0:1, 0:1])
            if mode in ("dma", "segsum"):
                res_t = big_pool.tile([P, d], f32, tag="res")
                nc.sync.dma_start(res_t[:], RES[g * P:(g + 1) * P, :])
                out_t = big_pool.tile([P, d], f32, tag="out")
                nc.vector.tensor_copy(
                    out_t[:], catT[:].rearrange("p k j -> p (k j)")[:, 0:d])
                nc.vector.tensor_add(out_t[:], out_t[:], res_t[:])
                nc.sync.dma_start(OUT[g * P:(g + 1) * P, :], out_t[:])
                return

            # ---- layer 1: hT[m] = W1[:, m]^T @ catT   (H^T, hid on partitions)
            hp = h_ps.tile([P, FT, P], f32)
            for m in range(FT):
                for k in range(KT):
                    nc.tensor.matmul(
                        hp[:, m, :],
                        w1_sb[:, k, m * P:(m + 1) * P],
                        catT[:, k, :],
                        start=(m == 0 and k == 0),
                        stop=(m == FT - 1 and k == KT - 1))
            # silu(x + b1) = (x + b1) * sigmoid(x + b1); keep f32 until product
            hT = h_pool.tile([P, FT, P], fmm)
            sg = h_pool.tile([P, FT, P], f32, tag="sig")
            hb = h_pool.tile([P, FT, P], f32, tag="hb")
            for m in range(FT):
                nc.scalar.activation(sg[:, m, :], hp[:, m, :], AF.Sigmoid,
                                     bias=b1_sb[:, m:m + 1], scale=1.0)
                nc.vector.tensor_scalar(hb[:, m, :], hp[:, m, :],
                                        b1_sb[:, m:m + 1], None, add)
                nc.vector.tensor_mul(hT[:, m, :], sg[:, m, :], hb[:, m, :])

            # ---- layer 2: y[n, f] = hT^T @ W2 + b2
            yp = y_ps.tile([P, d], f32)
            for k in range(FT):
                nc.tensor.matmul(yp[:], hT[:, k, :], w2_sb[:, k, :],
                                 start=(k == 0), stop=False)
            nc.tensor.matmul(yp[:], ones_sb[:1, :P], b2_sb[:1, :],
                             start=False, stop=True)

            # ---- layernorm + residual
            musum = stat_pool.tile([P, 1], f32, tag="musum")
            nc.vector.tensor_reduce(musum[:], yp[:], axis=mybir.AxisListType.X,
                                    op=add)
            negmu = stat_pool.tile([P, 1], f32, tag="negmu")
            nc.scalar.mul(negmu[:], musum[:], -1.0 / d)
            xc = big_pool.tile([P, d], f32, tag="xc")
            nc.scalar.activation(xc[:], yp[:], AF.Identity, bias=negmu[:],
                                 scale=1.0)
            sq = big_pool.tile([P, d], f32, tag="sq")
            varsum = stat_pool.tile([P, 1], f32, tag="varsum")
            nc.scalar.activation(sq[:], xc[:], AF.Square, accum_out=varsum[:])
            stdv = stat_pool.tile([P, 1], f32, tag="stdv")
            nc.scalar.activation(stdv[:], varsum[:], AF.Sqrt, bias=eps_sb[:],
                                 scale=1.0 / d)
            rstd = stat_pool.tile([P, 1], f32, tag="rstd")
            nc.vector.reciprocal(rstd[:], stdv[:])
            yn = big_pool.tile([P, d], f32, tag="yn")
            nc.vector.tensor_scalar(yn[:], xc[:], rstd[:], None, mult)
            res_t = big_pool.tile([P, d], f32, tag="res")
            nc.sync.dma_start(res_t[:], RES[g * P:(g + 1) * P, :])
            t2 = big_pool.tile([P, d], f32, tag="t2")
            nc.vector.tensor_mul(t2[:], yn[:], gam_sb[:])
            out_t = big_pool.tile([P, d], f32, tag="out")
            nc.vector.tensor_add(out_t[:], t2[:], res_t[:])
            nc.sync.dma_start(OUT[g * P:(g + 1) * P, :], out_t[:])

        def all_groups():
            for g in range(gpc):
                group_body(g)

        if reps > 1:
            with tc.For_i(0, reps, 1):
                all_groups()
        else:
            all_groups()

    nc.compile()
    return nc


# ---------------------------------------------------------------- entry point
def kernel(efeat, nfeat, dst_idx, w1, b1, w2, b2, gamma, beta):
    from concourse.bass_utils import run_bass_kernel_spmd

    efeat = np.asarray(efeat, dtype=np.float32)
    nfeat = np.asarray(nfeat, dtype=np.float32)
    dst_idx = np.asarray(dst_idx, dtype=np.int32)
    args = [np.asarray(a, dtype=np.float32)
            for a in (w1, b1, w2, b2, gamma, beta)]

    per_core, perm, cpg, gpc = pack_inputs(efeat, nfeat, dst_idx, *args)
    nc = build_nc(gpc, cpg, d=efeat.shape[1], reps=1)
    res = run_bass_kernel_spmd(nc, per_core, core_ids=list(range(N_CORES)))

    out_all = np.concatenate([res.results[c]["OUT"] for c in range(N_CORES)],
                             axis=0)
    valid = perm >= 0
    nfeat_new = np.empty_like(nfeat)
    nfeat_new[perm[valid]] = out_all[valid]
    return efeat, nfeat_new
